# revision 1
# baseline (speedup 1.0000x reference)
# Conv2d 3x3 SAME (stride 1) on Trainium2, data-parallel over batch on 8 cores.
#
# Full problem: x[16, 64, 256, 256] f32, weight[128, 64, 3, 3], bias[128]
#   -> out[16, 128, 256, 256] f32.
#
# Per-core kernel (2 images/core): conv lowered to shift-and-matmul, v3.
#
# Roofline (per core): 9 taps x 64ci x 128co x 256x256 x 2img = 9.66 G MAC
#   -> ~246us at the fp16 PE peak (dual-tile, both 64-row halves active).
#   HBM: x fp16 16.8MB + y fp16 33.6MB = 50MB -> ~141us at 358 GB/s.
#   The kernel should therefore be PE-bound. v1 moved 105MB (x read twice,
#   y in f32) and was HBM-bound at 311us; v2 fixed the traffic and ran the
#   PE 100% busy mid-kernel.
#
# Structure ("strip pair"):
#   - The dual-tile trick runs tap t for TWO independent 16-row strips
#     concurrently: strip A (output rows r0..r0+15) streams from SBUF
#     partitions 0..63 into PE rows 0..63 (tile_position (0,0)), strip B
#     (rows r0+16..r0+31) from partitions 64..127 (tile_position (64,0)).
#     Pairing two strips instead of two row-groups of one strip means each
#     half-strip of x is DMAd once, into one partition half — no duplicated
#     HBM read and no on-chip copy.
#   - B processes its groups rotated by +4 relative to A, so the two
#     concurrently-streaming rhs reads always sit at different SBUF byte
#     offsets. v2 ran A and B at identical offsets (different partition
#     halves) and every dual slot paid ~+48ns — same-address port conflict.
#   - Host pre-pads x -> xp[bpc, 64, 258, 258] fp16; a tap (kh, kw) is an
#     AP offset into the SBUF strip, no edge handling on device.
#   - PSUM accumulates 9 taps per 2-row group (N = 512, one bank). PSUM
#     evictions are fused with the bias add and the f32->fp16 convert:
#     psa on DVE (tensor_scalar_add), psb on ScalarE (activation Identity
#     with per-partition bias) so neither engine rides the critical path.
#   - y is stored fp16 (halves the dominant HBM stream; adds ~5e-4 rel
#     error vs the 2e-2 budget) and upcast to f32 on the host. ylo rides
#     the scalar HWDGE ring, yhi the sync ring, x loads the sync ring.
#   - A handful of warm-up matmuls run while the first x strips are in
#     flight so the PE HAM clock-gate (cold 1.2 GHz -> warm 2.4 GHz after
#     ~3.4us of sustained activity) is already released when real work
#     starts; image 0 opens and image 1 closes with 8-row half-pairs so
#     the first strip load is small (window opens ~2us sooner) and the
#     final y stores are small (drain ~2us shorter).
#
# Measured: ~269-270us HW exec (PE window ~250us = 1152 dual slots x
# 216.4ns, plus ~7.5us framework preamble, ~5us warmup bridge, ~4.3us
# store drain, ~6us end barrier). Note the chip drops PE 2.4->2.0 GHz
# (P0 power state) after sustained back-to-back benching: +20% whole-
# kernel; idle a few minutes and re-run before trusting an outlier.

import numpy as np

import concourse.bass as bass
import concourse.mybir as mybir
import concourse.tile as tile
from concourse import bacc
from concourse.bass_utils import run_bass_kernel_spmd

N_CORES = 8
B, C_IN, H, W = 16, 64, 256, 256
C_OUT = 128
BPC = B // N_CORES  # images per core

F16 = mybir.dt.float16
F32 = mybir.dt.float32

SROWS = 16  # output rows per half-strip (one partition half)
GR = 2  # output rows per PSUM group (N = GR*W = 512; one 2KB bank)
N_WARMUP = 26  # bridge PE activity from preamble end (~7.8us) until
# the first (small) strip pair has landed (~11us), so the HAM
# throttle releases before real work and never re-arms


def build_nc(bpc=BPC, h=H, w=W, gr=GR):
    """Per-core Bass module. Input xp is the host-padded image
    [bpc, C_IN, h+2, w+2] (zero border), fp16."""
    assert h % (2 * SROWS) == 0 and SROWS % gr == 0
    ng = SROWS // gr  # groups per half-strip
    wp = w + 2
    xrows = SROWS + 2  # row slots per half-strip
    nc = bacc.Bacc("TRN2", target_bir_lowering=False, debug=False)

    xp_d = nc.dram_tensor("xp", [bpc, C_IN, h + 2, wp], F16, kind="ExternalInput")
    # all 9 taps, replicated into both partition halves: [2*C_IN, 9, C_OUT]
    wall_d = nc.dram_tensor("wall", [2 * C_IN, 9, C_OUT], F16, kind="ExternalInput")
    bias_d = nc.dram_tensor("bias", [C_OUT, 1], F32, kind="ExternalInput")
    y_d = nc.dram_tensor("y", [bpc, C_OUT, h, w], F16, kind="ExternalOutput")

    with tile.TileContext(nc) as tc:
        with (
            tc.tile_pool(name="consts", bufs=1) as consts,
            tc.tile_pool(name="xpool", bufs=2) as xpool,
            tc.tile_pool(name="ypool", bufs=4) as ypool,
            # psa/psb tags x 3 bufs = 6 banks, + 2 warm-up banks = 8 of 8
            tc.tile_pool(name="psum", bufs=3, space="PSUM") as psum,
            tc.tile_pool(name="warm", bufs=1, space="PSUM") as warm,
        ):
            # consts ride the scalar ring: each dma_start pays ~1.5-2.8us of
            # serialized HWDGE setup on its ring, and the sync ring is the
            # critical path to the first strip landing — keep it x-only at
            # the start.
            wall_sb = consts.tile([2 * C_IN, 9, C_OUT], F16)
            nc.scalar.dma_start(out=wall_sb, in_=wall_d.ap())
            bias_sb = consts.tile([C_OUT, 1], F32)
            nc.scalar.dma_start(out=bias_sb, in_=bias_d.ap())

            # Warm-up: dummy matmuls on a memset scratch tile (no DMA
            # dependency) keep the PE busy from the very start, releasing
            # the HAM throttle (cold 1.2 GHz -> warm 2.4 GHz after ~3.4us
            # sustained) before the first real matmul. They must alternate
            # two independent PSUM tiles on the two array halves to issue
            # back-to-back — a single accumulation target serializes on
            # WAW and the resulting drain gaps never sustain the HAM
            # window. Results never read.
            scratch = consts.tile([128, gr * w], F16)
            nc.gpsimd.memset(scratch[:, :], 0)
            wps_a = warm.tile([C_OUT, gr * w], F32, tag="warm_a")
            wps_b = warm.tile([C_OUT, gr * w], F32, tag="warm_b")
            for i in range(N_WARMUP):
                nc.tensor.matmul(
                    wps_a if i % 2 == 0 else wps_b,
                    lhsT=scratch[0:C_IN, 0:C_OUT] if i % 2 == 0 else scratch[C_IN:128, 0:C_OUT],
                    rhs=scratch[0:C_IN, :] if i % 2 == 0 else scratch[C_IN:128, :],
                    start=True,
                    stop=True,
                    tile_position=(0, 0) if i % 2 == 0 else (64, 0),
                )

            # Image 0 starts with two 8-row pairs (the first x load is
            # 0.66MB instead of 1.18MB, so the real matmul window opens
            # ~2us sooner); image 1 ends with two 8-row pairs (the final y
            # stores halve, shortening the end-of-kernel drain). Remaining
            # rows in full-size pairs.
            sm = SROWS // 2
            pairs_first = [(0, sm), (SROWS, sm)] + [
                (r, SROWS) for r in range(2 * SROWS, h, 2 * SROWS)
            ]
            pairs_last = [(r, SROWS) for r in range(0, h - 2 * SROWS, 2 * SROWS)] + [
                (h - 2 * SROWS, sm), (h - SROWS, sm)
            ]
            for n in range(bpc):
                for r0, srows in pairs_first if n == 0 else pairs_last:
                    r1 = r0 + srows
                    ngp = srows // gr
                    xrows_p = srows + 2
                    # slot s of the lower half <-> padded row r0+s; of the
                    # upper half <-> padded row r1+s. Output row j reads
                    # padded rows j..j+2, so group row ja+i tap kh is slot
                    # ja+i+kh — max srows-2+1+2 < srows+2 slots.
                    xl = xpool.tile([128, xrows_p, wp], F16, tag=f"xl{srows}")
                    nc.sync.dma_start(
                        out=xl[0:C_IN, :, :],
                        in_=xp_d.ap()[n, :, r0 : r0 + xrows_p, :],
                    )
                    nc.sync.dma_start(
                        out=xl[C_IN:128, :, :],
                        in_=xp_d.ap()[n, :, r1 : r1 + xrows_p, :],
                    )

                    ylo = ypool.tile([C_OUT, srows, w], F16, tag=f"ylo{srows}")
                    yhi = ypool.tile([C_OUT, srows, w], F16, tag=f"yhi{srows}")
                    half = srows // 2
                    for g in range(ngp):
                        ja = gr * g
                        jb = gr * ((g + ngp // 2) % ngp)  # B rotated: offsets differ
                        psa = psum.tile([C_OUT, gr, w], F32, tag="psa")
                        psb = psum.tile([C_OUT, gr, w], F32, tag="psb")
                        for t in range(9):
                            kh, kw = divmod(t, 3)
                            nc.tensor.matmul(
                                psa,
                                lhsT=wall_sb[0:C_IN, t, :],
                                rhs=xl[0:C_IN, ja + kh : ja + kh + gr, kw : kw + w],
                                start=(t == 0),
                                stop=(t == 8),
                                tile_position=(0, 0),
                            )
                            nc.tensor.matmul(
                                psb,
                                lhsT=wall_sb[C_IN:128, t, :],
                                rhs=xl[C_IN:128, jb + kh : jb + kh + gr, kw : kw + w],
                                start=(t == 0),
                                stop=(t == 8),
                                tile_position=(64, 0),
                            )
                        nc.vector.tensor_scalar_add(ylo[:, ja : ja + gr, :], psa, bias_sb)
                        nc.scalar.activation(
                            yhi[:, jb : jb + gr, :],
                            psb,
                            mybir.ActivationFunctionType.Identity,
                            bias=bias_sb,
                        )
                        # store each finished 8-row half as soon as its last
                        # eviction lands: A fills ylo rows in order, B fills
                        # yhi rows 8..16 first (rotation), then 0..8.
                        if g == ngp // 2 - 1:
                            nc.scalar.dma_start(
                                out=y_d.ap()[n, :, r0 : r0 + half, :],
                                in_=ylo[:, 0:half, :],
                            )
                            nc.sync.dma_start(
                                out=y_d.ap()[n, :, r1 + half : r1 + srows, :],
                                in_=yhi[:, half:srows, :],
                            )
                        elif g == ngp - 1:
                            nc.scalar.dma_start(
                                out=y_d.ap()[n, :, r0 + half : r0 + srows, :],
                                in_=ylo[:, half:srows, :],
                            )
                            nc.sync.dma_start(
                                out=y_d.ap()[n, :, r1 : r1 + half, :],
                                in_=yhi[:, 0:half, :],
                            )


    nc.compile()
    return nc


def pad_x(x):
    """[n, c, h, w] -> zero-bordered fp16 [n, c, h+2, w+2]."""
    n, c, h, w = x.shape
    xp = np.zeros((n, c, h + 2, w + 2), np.float16)
    xp[:, :, 1 : h + 1, 1 : w + 1] = x
    return xp


def prep_weights(weight):
    """weight [C_OUT, C_IN, 3, 3] -> lhsT layout [2*ci, tap, co]."""
    wt = np.ascontiguousarray(np.transpose(weight, (1, 2, 3, 0)).astype(np.float16))
    w9 = wt.reshape(C_IN, 9, C_OUT)
    return np.ascontiguousarray(np.concatenate([w9, w9], axis=0))


_NC_CACHE = {}
LAST_RESULT = None  # BassKernelResults of the most recent run (for test harness)
TRACE = False


def kernel(x, weight, bias):
    global LAST_RESULT
    x = np.asarray(x, dtype=np.float32)
    weight = np.asarray(weight, dtype=np.float32)
    bias = np.asarray(bias, dtype=np.float32)

    key = ("v3", GR)
    if key not in _NC_CACHE:
        _NC_CACHE[key] = build_nc()
    nc = _NC_CACHE[key]

    xp = pad_x(x)
    wall = prep_weights(weight)
    bias2 = np.ascontiguousarray(bias.reshape(C_OUT, 1))

    in_maps = []
    for c in range(N_CORES):
        in_maps.append(
            {
                "xp": xp[c * BPC : (c + 1) * BPC],
                "wall": wall,
                "bias": bias2,
            }
        )

    res = run_bass_kernel_spmd(nc, in_maps, core_ids=list(range(N_CORES)), trace=TRACE)
    LAST_RESULT = res
    out = np.concatenate([r["y"] for r in res.results], axis=0).astype(np.float32)
    return out



# revision 3
# speedup vs baseline: 1.0192x; 1.0192x over previous
# Conv2d 3x3 SAME (stride 1) on Trainium2, data-parallel over batch on 8 cores.
#
# Full problem: x[16, 64, 256, 256] f32, weight[128, 64, 3, 3], bias[128]
#   -> out[16, 128, 256, 256] f32.
#
# Per-core kernel (2 images/core): conv lowered to shift-and-matmul, v4.
#
# v3 recap (271us): 9 taps x fp16 matmuls, dual-tile (two independent 16-row
#   strips on the two 64-row PE halves), PSUM accumulates the 9 taps per
#   2-row group (N = 512), evictions fused with bias add + f32->fp16 on
#   DVE/ScalarE, y stored fp16. In-window the PE ran at the fp16 peak
#   (216.4ns per dual slot vs 215.8 theoretical), so the only remaining
#   lever is the fp8 DoubleRow matmul mode (2 fp8 weights/cell, ~1.44x).
#
# v4 ("S2 hybrid"): taps (0,0) and (0,1) move to ONE fp8e4 DoubleRow
#   matmul per group-half; the other 7 taps stay fp16. Numerics (fixed
#   seed, simulated vs CPU reference): full fp8 rel=0.037, 4-tap hybrid
#   0.025 -- both over the 2e-2 budget; 2-tap hybrid = 0.0184 < 2e-2.
#   Speed: DR slot ~= 518*1.13 cycles vs 2 fp16 slots, saving ~188 cycles
#   per group-half -> predicted window 249 -> ~225us, total ~247us.
#
#   DoubleRow operand shapes: lhsT [K, 2, M] fp8, rhs [K, 2, N] fp8,
#   out [M, N] += sum_p lhsT[:,p,:].T @ rhs[:,p,:]. To keep the moving AP
#   3D with HW-friendly steps, the host pre-interleaves the fp8 image:
#     xq8i[n, ci, row, c, p] = pad(x)[n, ci, row, c+p],  c in 0..255, p in 0..1
#   stored as rows of exactly 512 bytes. The group's moving AP is then
#   [K=64, pair(2, step 1), N(512, step 2)] -- the (row, col) walk merges
#   because the row span equals the row pitch. Weight pairs sit at step
#   128 B (co-contiguous), satisfying the DoubleRow step%16==0 rule.
#
# Structure (unchanged from v3 otherwise):
#   - strip pair: lower half-strip in partitions 0..63 (tile_position
#     (0,0)), upper in 64..127 ((64,0)); B processes groups rotated +4 so
#     concurrent rhs streams never sit at the same SBUF byte offset.
#   - psa evictions on DVE (tensor_scalar_add w/ bias), psb on ScalarE
#     (activation Identity w/ bias); y fp16, upcast on host.
#   - x16 + yhi ride the sync HWDGE ring; consts + x8 + ylo the scalar
#     ring (~34MB each side, 67MB total = ~188us of HBM, under the
#     ~225us PE window).
#   - warm-up matmuls release the PE HAM clock-gate before real work;
#     image 0 opens / image 1 closes with 8-row half-pairs.
#
# Measured: see test.py runs; v3 baseline 270728ns.

import numpy as np
import ml_dtypes

import concourse.bass as bass
import concourse.mybir as mybir
import concourse.tile as tile
from concourse import bacc
from concourse.bass_utils import run_bass_kernel_spmd

N_CORES = 8
B, C_IN, H, W = 16, 64, 256, 256
C_OUT = 128
BPC = B // N_CORES  # images per core

F8 = mybir.dt.float8e4
F16 = mybir.dt.float16
F32 = mybir.dt.float32
NP_F8 = ml_dtypes.float8_e4m3fn

SROWS = 16  # output rows per half-strip (one partition half)
GR = 2  # output rows per PSUM group (N = GR*W = 512; one 2KB bank)
N_WARMUP = 26

# 7 fp16 taps; taps (0,0),(0,1) are handled by the DoubleRow pair
FP16_TAPS = [(0, 2), (1, 0), (1, 1), (1, 2), (2, 0), (2, 1), (2, 2)]


def build_nc(bpc=BPC, h=H, w=W, gr=GR):
    """Per-core Bass module. Inputs: xp   host-padded fp16 [bpc, C_IN, h+2, w+2],
    xq8i pair-interleaved fp8 [bpc, C_IN, h+2, 2*w] (see header)."""
    assert h % (2 * SROWS) == 0 and SROWS % gr == 0
    wp = w + 2
    nc = bacc.Bacc("TRN2", target_bir_lowering=False, debug=False)

    xp_d = nc.dram_tensor("xp", [bpc, C_IN, h + 2, wp], F16, kind="ExternalInput")
    x8_d = nc.dram_tensor("x8", [bpc, C_IN, h + 2, 2 * w], F8, kind="ExternalInput")
    # fp16 taps, replicated into both partition halves: [2*C_IN, 7, C_OUT]
    wall_d = nc.dram_tensor("wall", [2 * C_IN, len(FP16_TAPS), C_OUT], F16,
                            kind="ExternalInput")
    # DoubleRow pair taps (0,0),(0,1): [2*C_IN, 2, C_OUT] fp8
    w8_d = nc.dram_tensor("w8", [2 * C_IN, 2, C_OUT], F8, kind="ExternalInput")
    bias_d = nc.dram_tensor("bias", [C_OUT, 1], F32, kind="ExternalInput")
    y_d = nc.dram_tensor("y", [bpc, C_OUT, h, w], F16, kind="ExternalOutput")

    with tile.TileContext(nc) as tc:
        with (
            tc.tile_pool(name="consts", bufs=1) as consts,
            tc.tile_pool(name="xpool", bufs=2) as xpool,
            tc.tile_pool(name="x8pool", bufs=2) as x8pool,
            tc.tile_pool(name="ypool", bufs=4) as ypool,
            # psa/psb tags x 3 bufs = 6 banks, + 2 warm-up banks = 8 of 8
            tc.tile_pool(name="psum", bufs=3, space="PSUM") as psum,
            tc.tile_pool(name="warm", bufs=1, space="PSUM") as warm,
        ):
            # consts ride the scalar ring; keep the sync ring x-only at the
            # start (it is the critical path to the first strip landing).
            wall_sb = consts.tile([2 * C_IN, len(FP16_TAPS), C_OUT], F16)
            nc.scalar.dma_start(out=wall_sb, in_=wall_d.ap())
            w8_sb = consts.tile([2 * C_IN, 2, C_OUT], F8)
            nc.scalar.dma_start(out=w8_sb, in_=w8_d.ap())
            bias_sb = consts.tile([C_OUT, 1], F32)
            nc.scalar.dma_start(out=bias_sb, in_=bias_d.ap())

            # Warm-up: dummy matmuls on a memset scratch tile keep the PE
            # busy from the start, releasing the HAM throttle (cold 1.2GHz
            # -> warm 2.4GHz after ~3.4us sustained) before the first real
            # matmul. Two PSUM tiles on the two array halves so they issue
            # back-to-back. Results never read.
            scratch = consts.tile([128, gr * w], F16)
            nc.gpsimd.memset(scratch[:, :], 0)
            wps_a = warm.tile([C_OUT, gr * w], F32, tag="warm_a")
            wps_b = warm.tile([C_OUT, gr * w], F32, tag="warm_b")
            for i in range(N_WARMUP):
                nc.tensor.matmul(
                    wps_a if i % 2 == 0 else wps_b,
                    lhsT=scratch[0:C_IN, 0:C_OUT] if i % 2 == 0 else scratch[C_IN:128, 0:C_OUT],
                    rhs=scratch[0:C_IN, :] if i % 2 == 0 else scratch[C_IN:128, :],
                    start=True,
                    stop=True,
                    tile_position=(0, 0) if i % 2 == 0 else (64, 0),
                )

            # Image 0 opens with two 8-row pairs (smaller first x load);
            # image 1 closes with two 8-row pairs (smaller final y stores).
            sm = SROWS // 2
            pairs_first = [(0, sm), (SROWS, sm)] + [
                (r, SROWS) for r in range(2 * SROWS, h, 2 * SROWS)
            ]
            pairs_last = [(r, SROWS) for r in range(0, h - 2 * SROWS, 2 * SROWS)] + [
                (h - 2 * SROWS, sm), (h - SROWS, sm)
            ]

            for n in range(bpc):
                for r0, srows in pairs_first if n == 0 else pairs_last:
                    r1 = r0 + srows
                    ngp = srows // gr
                    xrows_p = srows + 2
                    # fp16 strip: slot s of the lower half <-> padded row
                    # r0+s; upper half <-> r1+s. Tap kh of group row ja+i is
                    # slot ja+i+kh.
                    xl = xpool.tile([128, xrows_p, wp], F16, tag=f"xl{srows}")
                    nc.sync.dma_start(
                        out=xl[0:C_IN, :, :],
                        in_=xp_d.ap()[n, :, r0 : r0 + xrows_p, :],
                    )
                    nc.sync.dma_start(
                        out=xl[C_IN:128, :, :],
                        in_=xp_d.ap()[n, :, r1 : r1 + xrows_p, :],
                    )
                    # fp8 interleaved strip: kh=0 only -> srows row slots
                    xl8 = x8pool.tile([128, srows, 2 * w], F8, tag=f"x8{srows}")
                    nc.scalar.dma_start(
                        out=xl8[0:C_IN, :, :],
                        in_=x8_d.ap()[n, :, r0 : r0 + srows, :],
                    )
                    nc.scalar.dma_start(
                        out=xl8[C_IN:128, :, :],
                        in_=x8_d.ap()[n, :, r1 : r1 + srows, :],
                    )

                    ylo = ypool.tile([C_OUT, srows, w], F16, tag=f"ylo{srows}")
                    yhi = ypool.tile([C_OUT, srows, w], F16, tag=f"yhi{srows}")
                    half = srows // 2
                    for g in range(ngp):
                        ja = gr * g
                        jb = gr * ((g + ngp // 2) % ngp)  # B rotated: offsets differ
                        psa = psum.tile([C_OUT, gr, w], F32, tag="psa")
                        psb = psum.tile([C_OUT, gr, w], F32, tag="psb")

                        # DoubleRow slot: taps (0,0)+(0,1), zeroes the bank
                        for half_sel, ps, jx in ((0, psa, ja), (1, psb, jb)):
                            p0 = half_sel * C_IN
                            sl = xl8[p0 : p0 + C_IN, jx : jx + gr, 0 : 2 * w]
                            rhs8 = bass.AP(
                                sl.tensor, sl.offset,
                                [list(sl.ap[0]), [1, 2], [2, gr * w]],
                            )
                            nc.tensor.matmul(
                                ps,
                                lhsT=w8_sb[p0 : p0 + C_IN, :, :],
                                rhs=rhs8,
                                start=True,
                                stop=False,
                                perf_mode=mybir.MatmulPerfMode.DoubleRow,
                                tile_position=(p0, 0),
                            )
                        # 7 fp16 taps accumulate on top
                        for ti, (kh, kw) in enumerate(FP16_TAPS):
                            last = ti == len(FP16_TAPS) - 1
                            nc.tensor.matmul(
                                psa,
                                lhsT=wall_sb[0:C_IN, ti, :],
                                rhs=xl[0:C_IN, ja + kh : ja + kh + gr, kw : kw + w],
                                start=False,
                                stop=last,
                                tile_position=(0, 0),
                            )
                            nc.tensor.matmul(
                                psb,
                                lhsT=wall_sb[C_IN:128, ti, :],
                                rhs=xl[C_IN:128, jb + kh : jb + kh + gr, kw : kw + w],
                                start=False,
                                stop=last,
                                tile_position=(64, 0),
                            )
                        nc.vector.tensor_scalar_add(ylo[:, ja : ja + gr, :], psa, bias_sb)
                        nc.scalar.activation(
                            yhi[:, jb : jb + gr, :],
                            psb,
                            mybir.ActivationFunctionType.Identity,
                            bias=bias_sb,
                        )
                        # store each finished 8-row half as soon as its last
                        # eviction lands: A fills ylo rows in order, B fills
                        # yhi rows 8..16 first (rotation), then 0..8.
                        if g == ngp // 2 - 1:
                            nc.scalar.dma_start(
                                out=y_d.ap()[n, :, r0 : r0 + half, :],
                                in_=ylo[:, 0:half, :],
                            )
                            nc.sync.dma_start(
                                out=y_d.ap()[n, :, r1 + half : r1 + srows, :],
                                in_=yhi[:, half:srows, :],
                            )
                        elif g == ngp - 1:
                            nc.scalar.dma_start(
                                out=y_d.ap()[n, :, r0 + half : r0 + srows, :],
                                in_=ylo[:, half:srows, :],
                            )
                            nc.sync.dma_start(
                                out=y_d.ap()[n, :, r1 : r1 + half, :],
                                in_=yhi[:, 0:half, :],
                            )

    nc.compile()
    return nc


def pad_x(x):
    """[n, c, h, w] -> zero-bordered fp16 [n, c, h+2, w+2]."""
    n, c, h, w = x.shape
    xp = np.zeros((n, c, h + 2, w + 2), np.float16)
    xp[:, :, 1 : h + 1, 1 : w + 1] = x
    return xp


def interleave_x8(x):
    """[n, c, h, w] f32 -> pair-interleaved fp8 [n, c, h+2, 2*w]:
    out[n,c,r,2*cc+p] = pad(x)[n,c,r,cc+p] for cc in 0..w-1, p in 0..1."""
    n, c, h, w = x.shape
    xp8 = np.zeros((n, c, h + 2, w + 2), NP_F8)
    xp8[:, :, 1 : h + 1, 1 : w + 1] = x.astype(NP_F8)
    out = np.empty((n, c, h + 2, w, 2), NP_F8)
    out[..., 0] = xp8[:, :, :, 0:w]
    out[..., 1] = xp8[:, :, :, 1 : w + 1]
    return np.ascontiguousarray(out.reshape(n, c, h + 2, 2 * w))


def prep_weights(weight):
    """weight [C_OUT, C_IN, 3, 3] -> fp16 lhsT [2*ci, 7, co] for FP16_TAPS
    and fp8 pair lhsT [2*ci, 2, co] for taps (0,0),(0,1)."""
    wt = np.transpose(weight, (1, 2, 3, 0))  # [ci, kh, kw, co]
    w16 = np.stack([wt[:, kh, kw, :] for (kh, kw) in FP16_TAPS], axis=1)
    w16 = np.ascontiguousarray(w16.astype(np.float16))
    w16 = np.ascontiguousarray(np.concatenate([w16, w16], axis=0))
    w8 = np.ascontiguousarray(wt[:, 0, 0:2, :].astype(NP_F8))  # [ci, 2, co]
    w8 = np.ascontiguousarray(np.concatenate([w8, w8], axis=0))
    return w16, w8


_NC_CACHE = {}
LAST_RESULT = None  # BassKernelResults of the most recent run (for test harness)
TRACE = False


def kernel(x, weight, bias):
    global LAST_RESULT
    x = np.asarray(x, dtype=np.float32)
    weight = np.asarray(weight, dtype=np.float32)
    bias = np.asarray(bias, dtype=np.float32)

    key = ("v4", GR)
    if key not in _NC_CACHE:
        _NC_CACHE[key] = build_nc()
    nc = _NC_CACHE[key]

    xp = pad_x(x)
    x8 = interleave_x8(x)
    w16, w8 = prep_weights(weight)
    bias2 = np.ascontiguousarray(bias.reshape(C_OUT, 1))

    in_maps = []
    for c in range(N_CORES):
        in_maps.append(
            {
                "xp": xp[c * BPC : (c + 1) * BPC],
                "x8": x8[c * BPC : (c + 1) * BPC],
                "wall": w16,
                "w8": w8,
                "bias": bias2,
            }
        )

    res = run_bass_kernel_spmd(nc, in_maps, core_ids=list(range(N_CORES)), trace=TRACE)
    LAST_RESULT = res
    out = np.concatenate([r["y"] for r in res.results], axis=0).astype(np.float32)
    return out


# revision 5
# speedup vs baseline: 1.0437x; 1.0241x over previous
# Conv2d 3x3 SAME (stride 1) on Trainium2, data-parallel over batch on 8 cores.
#
# Full problem: x[16, 64, 256, 256] f32, weight[128, 64, 3, 3], bias[128]
#   -> out[16, 128, 256, 256] f32.
#
# Per-core kernel (2 images/core): conv lowered to shift-and-matmul, v4.
#
# v3 recap (271us): 9 taps x fp16 matmuls, dual-tile (two independent 16-row
#   strips on the two 64-row PE halves), PSUM accumulates the 9 taps per
#   2-row group (N = 512), evictions fused with bias add + f32->fp16 on
#   DVE/ScalarE, y stored fp16. In-window the PE ran at the fp16 peak
#   (216.4ns per dual slot vs 215.8 theoretical), so the only remaining
#   lever is the fp8 DoubleRow matmul mode (2 fp8 weights/cell, ~1.44x).
#
# v4 ("S2 hybrid"): taps (0,0) and (0,1) move to ONE fp8e4 DoubleRow
#   matmul per group-half; the other 7 taps stay fp16. Numerics (fixed
#   seed, simulated vs CPU reference): full fp8 rel=0.037, 4-tap hybrid
#   0.025 -- both over the 2e-2 budget; 2-tap hybrid = 0.0184 < 2e-2.
#   Speed: DR slot ~= 518*1.13 cycles vs 2 fp16 slots, saving ~188 cycles
#   per group-half -> predicted window 249 -> ~225us, total ~247us.
#
#   DoubleRow operand shapes: lhsT [K, 2, M] fp8, rhs [K, 2, N] fp8,
#   out [M, N] += sum_p lhsT[:,p,:].T @ rhs[:,p,:]. To keep the moving AP
#   3D with HW-friendly steps, the host pre-interleaves the fp8 image:
#     xq8i[n, ci, row, c, p] = pad(x)[n, ci, row, c+p],  c in 0..255, p in 0..1
#   stored as rows of exactly 512 bytes. The group's moving AP is then
#   [K=64, pair(2, step 1), N(512, step 2)] -- the (row, col) walk merges
#   because the row span equals the row pitch. Weight pairs sit at step
#   128 B (co-contiguous), satisfying the DoubleRow step%16==0 rule.
#
# Structure (unchanged from v3 otherwise):
#   - strip pair: lower half-strip in partitions 0..63 (tile_position
#     (0,0)), upper in 64..127 ((64,0)); B processes groups rotated +4 so
#     concurrent rhs streams never sit at the same SBUF byte offset.
#   - psa evictions on DVE (tensor_scalar_add w/ bias), psb on ScalarE
#     (activation Identity w/ bias); y fp16, upcast on host.
#   - x16 + yhi ride the sync HWDGE ring; consts + x8 + ylo the scalar
#     ring (~34MB each side, 67MB total = ~188us of HBM, under the
#     ~225us PE window).
#   - warm-up matmuls release the PE HAM clock-gate before real work;
#     image 0 opens / image 1 closes with 8-row half-pairs.
#
# Measured: see test.py runs; v3 baseline 270728ns.

import numpy as np
import ml_dtypes

import concourse.bass as bass
import concourse.mybir as mybir
import concourse.tile as tile
from concourse import bacc
from concourse.bass_utils import run_bass_kernel_spmd

N_CORES = 8
B, C_IN, H, W = 16, 64, 256, 256
C_OUT = 128
BPC = B // N_CORES  # images per core

F8 = mybir.dt.float8e4
F16 = mybir.dt.float16
F32 = mybir.dt.float32
NP_F8 = ml_dtypes.float8_e4m3fn

SROWS = 16  # output rows per half-strip (one partition half)
GR = 2  # output rows per PSUM group (N = GR*W = 512; one 2KB bank)
N_WARMUP = 26

# 7 fp16 taps; taps (0,0),(0,1) are handled by the DoubleRow pair
FP16_TAPS = [(0, 2), (1, 0), (1, 1), (1, 2), (2, 0), (2, 1), (2, 2)]


def build_nc(bpc=BPC, h=H, w=W, gr=GR):
    """Per-core Bass module. Inputs: xp   host-padded fp16 [bpc, C_IN, h+2, w+2],
    xq8i pair-interleaved fp8 [bpc, C_IN, h+2, 2*w] (see header)."""
    assert h % (2 * SROWS) == 0 and SROWS % gr == 0
    wp = w + 2
    nc = bacc.Bacc("TRN2", target_bir_lowering=False, debug=False)

    xp_d = nc.dram_tensor("xp", [bpc, C_IN, h + 2, wp], F16, kind="ExternalInput")
    x8_d = nc.dram_tensor("x8", [bpc, C_IN, h + 2, 2 * w], F8, kind="ExternalInput")
    # fp16 taps, replicated into both partition halves: [2*C_IN, 7, C_OUT]
    wall_d = nc.dram_tensor("wall", [2 * C_IN, len(FP16_TAPS), C_OUT], F16,
                            kind="ExternalInput")
    # DoubleRow pair taps (0,0),(0,1): [2*C_IN, 2, C_OUT] fp8
    w8_d = nc.dram_tensor("w8", [2 * C_IN, 2, C_OUT], F8, kind="ExternalInput")
    bias_d = nc.dram_tensor("bias", [C_OUT, 1], F32, kind="ExternalInput")
    y_d = nc.dram_tensor("y", [bpc, C_OUT, h, w], F16, kind="ExternalOutput")

    with tile.TileContext(nc) as tc:
        with (
            tc.tile_pool(name="consts", bufs=1) as consts,
            tc.tile_pool(name="xpool", bufs=3) as xpool,
            tc.tile_pool(name="x8pool", bufs=3) as x8pool,
            tc.tile_pool(name="ypool", bufs=4) as ypool,
            # psa/psb tags x 3 bufs = 6 banks, + 2 warm-up banks = 8 of 8
            tc.tile_pool(name="psum", bufs=3, space="PSUM") as psum,
            tc.tile_pool(name="warm", bufs=1, space="PSUM") as warm,
        ):
            # consts ride the scalar ring; keep the sync ring x-only at the
            # start (it is the critical path to the first strip landing).
            wall_sb = consts.tile([2 * C_IN, len(FP16_TAPS), C_OUT], F16)
            nc.scalar.dma_start(out=wall_sb, in_=wall_d.ap())
            w8_sb = consts.tile([2 * C_IN, 2, C_OUT], F8)
            nc.scalar.dma_start(out=w8_sb, in_=w8_d.ap())
            bias_sb = consts.tile([C_OUT, 1], F32)
            nc.scalar.dma_start(out=bias_sb, in_=bias_d.ap())

            # Warm-up: dummy matmuls on a memset scratch tile keep the PE
            # busy from the start, releasing the HAM throttle (cold 1.2GHz
            # -> warm 2.4GHz after ~3.4us sustained) before the first real
            # matmul. Two PSUM tiles on the two array halves so they issue
            # back-to-back. Results never read.
            scratch = consts.tile([128, gr * w], F16)
            nc.gpsimd.memset(scratch[:, :], 0)
            wps_a = warm.tile([C_OUT, gr * w], F32, tag="warm_a")
            wps_b = warm.tile([C_OUT, gr * w], F32, tag="warm_b")
            for i in range(N_WARMUP):
                nc.tensor.matmul(
                    wps_a if i % 2 == 0 else wps_b,
                    lhsT=scratch[0:C_IN, 0:C_OUT] if i % 2 == 0 else scratch[C_IN:128, 0:C_OUT],
                    rhs=scratch[0:C_IN, :] if i % 2 == 0 else scratch[C_IN:128, :],
                    start=True,
                    stop=True,
                    tile_position=(0, 0) if i % 2 == 0 else (64, 0),
                )

            # Image 0 opens with two 8-row pairs (smaller first x load);
            # image 1 closes with two 8-row pairs (smaller final y stores).
            sm = SROWS // 2
            pairs_first = [(0, sm), (SROWS, sm)] + [
                (r, SROWS) for r in range(2 * SROWS, h, 2 * SROWS)
            ]
            pairs_last = [(r, SROWS) for r in range(0, h - 2 * SROWS, 2 * SROWS)] + [
                (h - 2 * SROWS, sm), (h - SROWS, sm)
            ]

            for n in range(bpc):
                for r0, srows in pairs_first if n == 0 else pairs_last:
                    r1 = r0 + srows
                    ngp = srows // gr
                    xrows_p = srows + 2
                    # fp16 strip: slot s of the lower half <-> padded row
                    # r0+s; upper half <-> r1+s. Tap kh of group row ja+i is
                    # slot ja+i+kh.
                    xl = xpool.tile([128, xrows_p, wp], F16, tag=f"xl{srows}")
                    nc.sync.dma_start(
                        out=xl[0:C_IN, :, :],
                        in_=xp_d.ap()[n, :, r0 : r0 + xrows_p, :],
                    )
                    nc.sync.dma_start(
                        out=xl[C_IN:128, :, :],
                        in_=xp_d.ap()[n, :, r1 : r1 + xrows_p, :],
                    )
                    # fp8 interleaved strip: kh=0 only -> srows row slots
                    xl8 = x8pool.tile([128, srows, 2 * w], F8, tag=f"x8{srows}")
                    nc.gpsimd.dma_start(
                        out=xl8[0:C_IN, :, :],
                        in_=x8_d.ap()[n, :, r0 : r0 + srows, :],
                    )
                    nc.gpsimd.dma_start(
                        out=xl8[C_IN:128, :, :],
                        in_=x8_d.ap()[n, :, r1 : r1 + srows, :],
                    )

                    ylo = ypool.tile([C_OUT, srows, w], F16, tag=f"ylo{srows}")
                    yhi = ypool.tile([C_OUT, srows, w], F16, tag=f"yhi{srows}")
                    half = srows // 2
                    for g in range(ngp):
                        ja = gr * g
                        jb = gr * ((g + ngp // 2) % ngp)  # B rotated: offsets differ
                        psa = psum.tile([C_OUT, gr, w], F32, tag="psa")
                        psb = psum.tile([C_OUT, gr, w], F32, tag="psb")

                        # DoubleRow slot: taps (0,0)+(0,1), zeroes the bank
                        for half_sel, ps, jx in ((0, psa, ja), (1, psb, jb)):
                            p0 = half_sel * C_IN
                            sl = xl8[p0 : p0 + C_IN, jx : jx + gr, 0 : 2 * w]
                            rhs8 = bass.AP(
                                sl.tensor, sl.offset,
                                [list(sl.ap[0]), [1, 2], [2, gr * w]],
                            )
                            nc.tensor.matmul(
                                ps,
                                lhsT=w8_sb[p0 : p0 + C_IN, :, :],
                                rhs=rhs8,
                                start=True,
                                stop=False,
                                perf_mode=mybir.MatmulPerfMode.DoubleRow,
                                tile_position=(p0, 0),
                            )
                        # 7 fp16 taps accumulate on top
                        for ti, (kh, kw) in enumerate(FP16_TAPS):
                            last = ti == len(FP16_TAPS) - 1
                            nc.tensor.matmul(
                                psa,
                                lhsT=wall_sb[0:C_IN, ti, :],
                                rhs=xl[0:C_IN, ja + kh : ja + kh + gr, kw : kw + w],
                                start=False,
                                stop=last,
                                tile_position=(0, 0),
                            )
                            nc.tensor.matmul(
                                psb,
                                lhsT=wall_sb[C_IN:128, ti, :],
                                rhs=xl[C_IN:128, jb + kh : jb + kh + gr, kw : kw + w],
                                start=False,
                                stop=last,
                                tile_position=(64, 0),
                            )
                        nc.vector.tensor_scalar_add(ylo[:, ja : ja + gr, :], psa, bias_sb)
                        nc.scalar.activation(
                            yhi[:, jb : jb + gr, :],
                            psb,
                            mybir.ActivationFunctionType.Identity,
                            bias=bias_sb,
                        )
                        # store each finished 8-row half as soon as its last
                        # eviction lands: A fills ylo rows in order, B fills
                        # yhi rows 8..16 first (rotation), then 0..8.
                        if g == ngp // 2 - 1:
                            nc.scalar.dma_start(
                                out=y_d.ap()[n, :, r0 : r0 + half, :],
                                in_=ylo[:, 0:half, :],
                            )
                            nc.gpsimd.dma_start(
                                out=y_d.ap()[n, :, r1 + half : r1 + srows, :],
                                in_=yhi[:, half:srows, :],
                            )
                        elif g == ngp - 1:
                            nc.scalar.dma_start(
                                out=y_d.ap()[n, :, r0 + half : r0 + srows, :],
                                in_=ylo[:, half:srows, :],
                            )
                            nc.gpsimd.dma_start(
                                out=y_d.ap()[n, :, r1 : r1 + half, :],
                                in_=yhi[:, 0:half, :],
                            )

    nc.compile()
    return nc


def pad_x(x):
    """[n, c, h, w] -> zero-bordered fp16 [n, c, h+2, w+2]."""
    n, c, h, w = x.shape
    xp = np.zeros((n, c, h + 2, w + 2), np.float16)
    xp[:, :, 1 : h + 1, 1 : w + 1] = x
    return xp


def interleave_x8(x):
    """[n, c, h, w] f32 -> pair-interleaved fp8 [n, c, h+2, 2*w]:
    out[n,c,r,2*cc+p] = pad(x)[n,c,r,cc+p] for cc in 0..w-1, p in 0..1."""
    n, c, h, w = x.shape
    xp8 = np.zeros((n, c, h + 2, w + 2), NP_F8)
    xp8[:, :, 1 : h + 1, 1 : w + 1] = x.astype(NP_F8)
    out = np.empty((n, c, h + 2, w, 2), NP_F8)
    out[..., 0] = xp8[:, :, :, 0:w]
    out[..., 1] = xp8[:, :, :, 1 : w + 1]
    return np.ascontiguousarray(out.reshape(n, c, h + 2, 2 * w))


def prep_weights(weight):
    """weight [C_OUT, C_IN, 3, 3] -> fp16 lhsT [2*ci, 7, co] for FP16_TAPS
    and fp8 pair lhsT [2*ci, 2, co] for taps (0,0),(0,1)."""
    wt = np.transpose(weight, (1, 2, 3, 0))  # [ci, kh, kw, co]
    w16 = np.stack([wt[:, kh, kw, :] for (kh, kw) in FP16_TAPS], axis=1)
    w16 = np.ascontiguousarray(w16.astype(np.float16))
    w16 = np.ascontiguousarray(np.concatenate([w16, w16], axis=0))
    w8 = np.ascontiguousarray(wt[:, 0, 0:2, :].astype(NP_F8))  # [ci, 2, co]
    w8 = np.ascontiguousarray(np.concatenate([w8, w8], axis=0))
    return w16, w8


_NC_CACHE = {}
LAST_RESULT = None  # BassKernelResults of the most recent run (for test harness)
TRACE = False


def kernel(x, weight, bias):
    global LAST_RESULT
    x = np.asarray(x, dtype=np.float32)
    weight = np.asarray(weight, dtype=np.float32)
    bias = np.asarray(bias, dtype=np.float32)

    key = ("v4", GR)
    if key not in _NC_CACHE:
        _NC_CACHE[key] = build_nc()
    nc = _NC_CACHE[key]

    xp = pad_x(x)
    x8 = interleave_x8(x)
    w16, w8 = prep_weights(weight)
    bias2 = np.ascontiguousarray(bias.reshape(C_OUT, 1))

    in_maps = []
    for c in range(N_CORES):
        in_maps.append(
            {
                "xp": xp[c * BPC : (c + 1) * BPC],
                "x8": x8[c * BPC : (c + 1) * BPC],
                "wall": w16,
                "w8": w8,
                "bias": bias2,
            }
        )

    res = run_bass_kernel_spmd(nc, in_maps, core_ids=list(range(N_CORES)), trace=TRACE)
    LAST_RESULT = res
    out = np.concatenate([r["y"] for r in res.results], axis=0).astype(np.float32)
    return out
